# revision 35
# baseline (speedup 1.0000x reference)
"""Additive attention (B=4, Q=512, K=2048, D=256, H=64) on 8 TRN2 NeuronCores.

Strategy
--------
scores[b,q,k] = sum_h wv[h] * tanh(qp[b,q,h] + kp[b,k,h]); masked softmax over k;
out = attn @ values.  The dominant cost is tanh on ACT (the only transcendental
engine, 1 elem/lane/cycle).  Structural optimizations:

1. Masked keys (k >= valid_len) contribute exp(-1e6) == 0 exactly, so only
   ceil(L_b/64)*64 key columns per batch need any compute.  The host gathers
   just the valid key columns; the graph is specialized at runtime to the
   valid_lens actually received (all 8 cores share one graph; per-core
   variation is data only).

2. Layout h-on-partitions with TWO queries packed per 128-partition ACT call
   (partitions 0:64 = h for q-slot-A, 64:128 = h for q-slot-B, against a kp2
   tile holding kp.T duplicated in both halves).  The "+qp" broadcast add is
   free via ACT's per-partition bias operand.  The h-reduction runs on the PE:
   pair j multiplies with a (128,32) weight whose only nonzero columns are 2j
   (top-half wv) and 2j+1 (bottom-half wv), accumulating into a shared PSUM
   tile -> 16 pairs build 32 score rows at no extra PE streaming cost.

3. All matmul operands are bf16 (fp32 streams at half rate on the PE; PSUM
   accumulation stays fp32).  ACT is dtype-independent so tanh/exp lose
   nothing; precision impact ~1e-3 total.

Softmax needs no max-subtraction (|score| <= ||wv||_1 ~ 2.6) and no on-device
normalization: an appended ones-column in the values matrix yields sum(exp)
via the same value matmul, and the host divides.  Each query lives on exactly
one core (core c owns q[64c:64c+64) of every batch), so no merge is needed.
"""

import math

import numpy as np

B, Q, KK, D, H = 4, 512, 2048, 256, 64
P = 128
G = 64            # key gather granularity (columns)
CH = 128          # value/transpose chunk granularity
QPC = 64          # queries per (core, batch)
NCORES = 8
DPAD = D + 8      # values cols + [ones, 7*zero] padding

_GRAPH_CACHE: dict = {}


def _widths(n64s):
    Ws = [n * G for n in n64s]
    offs = np.concatenate([[0], np.cumsum(Ws)]).astype(int)
    kch = [(w + CH - 1) // CH for w in Ws]           # value chunks per region
    vbase = np.concatenate([[0], np.cumsum(kch)]).astype(int)
    return Ws, offs, kch, vbase


def _build_graph(n64s, repeat=1):
    """Build + compile the single-core SPMD graph for per-batch 64-col counts.

    repeat > 1 wraps the whole body in a hardware For_i loop — used only for
    wall-clock HW timing (per-iteration delta), never for actual results.
    """
    import contextlib

    import concourse.bass as bass
    import concourse.mybir as mybir
    from concourse import bacc
    from concourse.tile import TileContext

    f32 = mybir.dt.float32
    bf16 = mybir.dt.bfloat16
    AF = mybir.ActivationFunctionType
    Ws, offs, kch, vbase = _widths(n64s)
    C = int(sum(Ws))
    assert C > 0
    totvn = int(vbase[-1])
    regions = [b for b in range(B) if n64s[b] > 0]
    last_b = regions[-1]

    nc = bacc.Bacc("TRN2", target_bir_lowering=False, debug=False)
    # wqk2: cols 0:64 = Wq, 64:192 = [Wk|Wk]; wvsid: cols 0:512 = wv2s flat,
    # 512:640 = identity.  Combined tensors keep the serialized per-DMA issue
    # cost (~0.6us each) off the critical path.
    qw_e = nc.declare_dram_parameter("qw", [D, 4 * QPC + H + P], bf16,
                                     isOutput=False)
    kT_e = nc.declare_dram_parameter("keysT", [D, C], bf16, isOutput=False)
    va_e = nc.declare_dram_parameter("vaug", [totvn * CH, DPAD], bf16,
                                     isOutput=False)
    wvsid_e = nc.declare_dram_parameter("wvsid", [P, 16 * 32 + P], bf16,
                                        isOutput=False)
    out_e = nc.declare_dram_parameter("out", [P, 4 * 3 * QPC], f32, isOutput=True)

    with TileContext(nc) as tc:
        with (
            tc.tile_pool(name="const", bufs=1) as cpool,
            tc.tile_pool(name="big", bufs=1) as kpool,
            tc.tile_pool(name="feat", bufs=3) as fpool,
            tc.tile_pool(name="xsum", bufs=2) as xpool,
            tc.tile_pool(name="pexp", bufs=2) as ppool,
            tc.tile_pool(name="pts", bufs=4) as ptpool,
            tc.tile_pool(name="ps_mm", bufs=1, space="PSUM") as ps_mm,
            tc.tile_pool(name="ps_sc", bufs=1, space="PSUM") as ps_sc,
            tc.tile_pool(name="ps_tr", bufs=1, space="PSUM") as ps_tr,
            tc.tile_pool(name="ps_v", bufs=1, space="PSUM") as ps_v,
            tc.For_i(0, repeat, 1) if repeat > 1 else contextlib.nullcontext(),
        ):
            # ---- input loads, critical-path first
            # keysT in two tiles: region 0's cols land fast so kp2 chunk 0
            # (and the first tanh) start early; the rest is one big DMA.
            w1 = min(Ws[regions[0]], C)
            kt_a = kpool.tile([P, 2, w1], bf16)
            nc.sync.dma_start(kt_a[:], kT_e[:, 0:w1].rearrange("(c p) k -> p c k", p=P))
            # queries + [Wq | Wk | Wk] arrive as one combined per-core DMA
            qw_t = cpool.tile([P, 2, 4 * QPC + H + P], bf16)
            nc.sync.dma_start(qw_t[:], qw_e[:].rearrange("(c p) q -> p c q", p=P))
            qt_t = qw_t[:, :, 0:4 * QPC]
            wqk2_t = qw_t[:, :, 4 * QPC:]
            wvs_t = cpool.tile([P, 16, 32], bf16)
            nc.sync.dma_start(wvs_t[:], wvsid_e[:, 0:512].rearrange(
                "p (j c) -> p j c", j=16))
            kt_b = None
            if C > w1:
                kt_b = kpool.tile([P, 2, C - w1], bf16)
                nc.sync.dma_start(kt_b[:],
                                  kT_e[:, w1:C].rearrange("(c p) k -> p c k", p=P))
            id_t = cpool.tile([P, P], bf16)
            nc.sync.dma_start(id_t[:], wvsid_e[:, 512:512 + P])
            va_t = kpool.tile([P, totvn, DPAD], bf16)
            nc.sync.dma_start(va_t[:], va_e[:].rearrange("(n p) d -> p n d", p=P))

            # ---- kp2 = Wk2.T @ keysT  -> (128, C) in SBUF (both halves = kp.T)
            # 512-col slices are emitted lazily, right before the first region
            # that reads them, so PE score matmuls are not queued behind the
            # whole projection.
            kp2 = kpool.tile([P, C], bf16)
            qp2 = cpool.tile([P, QPC // 2 * 4], f32)

            def emit_kp2_chunk(ps_pool, c0, w, tag="kp"):
                if c0 < w1:
                    r0, r1 = kt_a[:, 0, c0:c0 + w], kt_a[:, 1, c0:c0 + w]
                else:
                    r0 = kt_b[:, 0, c0 - w1:c0 - w1 + w]
                    r1 = kt_b[:, 1, c0 - w1:c0 - w1 + w]
                pt = ps_pool.tile([P, 512], f32, tag=tag)
                nc.tensor.matmul(pt[:, :w], lhsT=wqk2_t[:, 0, H:H + P],
                                 rhs=r0, start=True, stop=False)
                nc.tensor.matmul(pt[:, :w], lhsT=wqk2_t[:, 1, H:H + P],
                                 rhs=r1, start=False, stop=True)
                nc.vector.tensor_copy(kp2[:, c0:c0 + w], pt[:, :w])

            # ---- per-batch regions
            # PSUM budget (8 banks): kp0 1 + kp 1 + sc 4 + tr/qp 1 + v 1
            if True:
                for cc in range(0, w1, 512):
                    emit_kp2_chunk(ps_mm, cc, min(512, w1 - cc), tag="kp0")

                # qp2 bias tile; pair j of batch b = (q_{64b+j}, q_{64b+32+j});
                # the two strided copies run on ACT, which idles before the
                # tanh stream anyway (keeps DVE off the critical path)
                qps = ps_tr.tile([H, 4 * QPC], f32, tag="tr")
                nc.tensor.matmul(qps[:], lhsT=wqk2_t[:, 0, 0:H], rhs=qt_t[:, 0, :],
                                 start=True, stop=False)
                nc.tensor.matmul(qps[:], lhsT=wqk2_t[:, 1, 0:H], rhs=qt_t[:, 1, :],
                                 start=False, stop=True)
                qps_r = qps[:].rearrange("h (b c) -> h b c", b=B)
                qp2_r = qp2[:].rearrange("p (b c) -> p b c", b=B)
                nc.scalar.copy(qp2_r[0:H], qps_r[:, :, 0:32])
                nc.scalar.copy(qp2_r[H:P], qps_r[:, :, 32:QPC])

                for b in regions:
                    W = Ws[b]
                    off = int(offs[b])
                    nch = kch[b]
                    if off >= w1:
                        for cc in range(off, off + W, 512):
                            emit_kp2_chunk(ps_mm, cc, min(512, off + W - cc))
                    sc = ps_sc.tile([QPC, W], f32, tag="sc")
                    # 4 pairs per tanh instruction: DVE precomputes kp2+qp for
                    # each pair (tensor_scalar_add with a per-partition scalar,
                    # 4x mode) so one wide ACT call amortizes the ~224-cycle
                    # per-instruction constant over 4 pairs.
                    NPAIR = 4
                    for kgrp in range(32 // NPAIR):
                        xg = xpool.tile([P, NPAIR * W], bf16, tag="xg")
                        for jj in range(NPAIR):
                            pi = NPAIR * kgrp + jj
                            nc.vector.tensor_scalar_add(
                                xg[:, jj * W:(jj + 1) * W], kp2[:, off:off + W],
                                qp2[:, 32 * b + pi:32 * b + pi + 1])
                        ftg = fpool.tile([P, NPAIR * W], bf16, tag="ft")
                        nc.scalar.activation(ftg[:], xg[:], AF.Tanh)
                        for jj in range(NPAIR):
                            pi = NPAIR * kgrp + jj
                            g, j = pi // 16, pi % 16
                            for c0 in range(0, W, 512):
                                w = min(512, W - c0)
                                nc.tensor.matmul(
                                    sc[32 * g:32 * g + 32, c0:c0 + w],
                                    lhsT=wvs_t[:, j, :],
                                    rhs=ftg[:, jj * W + c0:jj * W + c0 + w],
                                    start=(j == 0), stop=(j == 15))
                    pe = ppool.tile([QPC, W], bf16, tag="P")
                    if b == last_b:
                        # chunk-wise exp lets the tail chain start earlier
                        for ci in range(nch):
                            cw = min(CH, W - CH * ci)
                            nc.scalar.activation(pe[:, CH * ci:CH * ci + cw],
                                                 sc[:, CH * ci:CH * ci + cw], AF.Exp)
                    else:
                        nc.scalar.activation(pe[:], sc[:], AF.Exp)
                    # one PSUM bank holds all value accumulators:
                    # cols 0:64 = values[:,0:128].T @ P, 64:128 = values[:,128:256].T @ P,
                    # 128:192 (partitions 0:8) = [ones|pad].T @ P
                    vp = ps_v.tile([P, 3 * QPC], f32, tag="v")
                    for ci in range(nch):
                        cw = min(CH, W - CH * ci)
                        tp = ps_tr.tile([P, QPC], bf16, tag="tr")
                        nc.tensor.transpose(tp[0:cw, :], pe[:, CH * ci:CH * ci + cw],
                                            id_t[0:QPC, 0:QPC])
                        ptt = ptpool.tile([P, QPC], bf16, tag="pt")
                        nc.vector.tensor_copy(ptt[0:cw, :], tp[0:cw, :])
                        gi = int(vbase[b]) + ci
                        # start=True clears has_written for the WHOLE bank, so
                        # only the very first matmul into this bank may use it;
                        # later first-writes rely on overwrite-where-bit-clear.
                        st, sp = (ci == 0), (ci == nch - 1)
                        nc.tensor.matmul(vp[:, 0:QPC], lhsT=va_t[0:cw, gi, 0:128],
                                         rhs=ptt[0:cw, :], start=st, stop=sp,
                                         skip_group_check=True)
                        nc.tensor.matmul(vp[:, QPC:2 * QPC],
                                         lhsT=va_t[0:cw, gi, 128:256],
                                         rhs=ptt[0:cw, :], start=False, stop=sp,
                                         skip_group_check=True)
                        nc.tensor.matmul(vp[0:8, 2 * QPC:3 * QPC],
                                         lhsT=va_t[0:cw, gi, 256:264],
                                         rhs=ptt[0:cw, :], start=False, stop=sp,
                                         skip_group_check=True)
                    # stream this region's whole accumulator out as one DMA;
                    # the last region's copy runs on ACT (free by then) to cut
                    # the cross-engine hop out of the tail
                    stg = ptpool.tile([P, 3 * QPC], f32, tag="stg")
                    if b == last_b:
                        nc.scalar.copy(stg[:], vp[:])
                    else:
                        nc.vector.tensor_copy(stg[:], vp[:])
                    nc.sync.dma_start(out_e[:, 3 * QPC * b:3 * QPC * (b + 1)], stg[:])

    nc.compile()
    return nc


def _prep_maps(queries, keys, values, L, Wq, Wk, wv, n64s):
    import ml_dtypes
    bf = ml_dtypes.bfloat16
    Ws, offs, kch, vbase = _widths(n64s)
    C = int(sum(Ws))
    keysT = np.zeros((D, C), np.float32)
    vaug = np.zeros((int(vbase[-1]) * CH, DPAD), np.float32)
    for b in range(B):
        if n64s[b] == 0:
            continue
        n = Ws[b]
        lb = int(L[b])
        off = int(offs[b])
        voff = int(vbase[b]) * CH
        kb = keys[b, :n, :].copy()
        vb = values[b, :n, :].copy()
        kb[lb:] = 0.0
        vb[lb:] = 0.0
        keysT[:, off:off + n] = kb.T
        vaug[voff:voff + n, 0:D] = vb
        vaug[voff:voff + n, D] = (np.arange(n) < lb).astype(np.float32)
    wv2s = np.zeros((P, 16, 32), np.float32)
    for j in range(16):
        wv2s[0:H, j, 2 * j] = wv
        wv2s[H:P, j, 2 * j + 1] = wv
    wqk2 = np.concatenate([Wq, Wk, Wk], axis=1)            # (256, 192)
    wvsid = np.concatenate([wv2s.reshape(P, 512), np.eye(P, dtype=np.float32)],
                           axis=1)                          # (128, 640)
    keysT = keysT.astype(bf)
    vaug = vaug.astype(bf)
    wvsid = np.ascontiguousarray(wvsid.astype(bf))
    in_maps = []
    for c in range(NCORES):
        qw = np.zeros((D, 4 * QPC + H + P), np.float32)
        for b in range(B):
            qw[:, QPC * b:QPC * (b + 1)] = queries[b, c * QPC:(c + 1) * QPC, :].T
        qw[:, 4 * QPC:] = wqk2
        in_maps.append(dict(qw=np.ascontiguousarray(qw.astype(bf)), keysT=keysT,
                            vaug=vaug, wvsid=wvsid))
    return in_maps


def kernel(queries, keys, values, valid_lens, Wq, Wk, wv):
    from concourse.bass_utils import run_bass_kernel_spmd

    queries = np.ascontiguousarray(np.asarray(queries, dtype=np.float32))
    keys = np.ascontiguousarray(np.asarray(keys, dtype=np.float32))
    values = np.ascontiguousarray(np.asarray(values, dtype=np.float32))
    Wq = np.ascontiguousarray(np.asarray(Wq, dtype=np.float32))
    Wk = np.ascontiguousarray(np.asarray(Wk, dtype=np.float32))
    wv = np.ascontiguousarray(np.asarray(wv, dtype=np.float32))
    L = np.clip(np.asarray(valid_lens).astype(np.int64), 0, KK)
    n64s = tuple(int(math.ceil(int(l) / G)) for l in L)

    out = np.zeros((B, Q, D), np.float32)
    for b in range(B):
        if n64s[b] == 0:
            # softmax over all -1e6 scores is uniform over ALL keys
            out[b, :, :] = values[b].mean(axis=0)[None, :]

    if sum(n64s) > 0:
        if n64s not in _GRAPH_CACHE:
            _GRAPH_CACHE[n64s] = _build_graph(n64s)
        nc = _GRAPH_CACHE[n64s]
        in_maps = _prep_maps(queries, keys, values, L, Wq, Wk, wv, n64s)
        res = run_bass_kernel_spmd(nc, in_maps, list(range(NCORES)))
        perm = np.array([32 * (r % 2) + r // 2 for r in range(QPC)])
        for c in range(NCORES):
            o = res.results[c]["out"]  # (128, 4*192): per region [d0|d1|sums]
            for b in range(B):
                if n64s[b] == 0:
                    continue
                blk = o[:, 3 * QPC * b:3 * QPC * (b + 1)]   # (128, 192)
                vals = np.concatenate([blk[:, 0:QPC], blk[:, QPC:2 * QPC]],
                                      axis=0)                # (256, 64)
                sums = blk[0, 2 * QPC:3 * QPC]               # (64,)
                out[b, c * QPC + perm, :] = (vals / sums[None, :]).T
    return out


# revision 37
# speedup vs baseline: 1.5394x; 1.5394x over previous
"""Additive attention (B=4, Q=512, K=2048, D=256, H=64) on 8 TRN2 NeuronCores.

Strategy
--------
scores[b,q,k] = sum_h wv[h] * tanh(qp[b,q,h] + kp[b,k,h]); masked softmax over k;
out = attn @ values.  The dominant cost is tanh on ACT (the only transcendental
engine, 1 elem/lane/cycle).  Structural optimizations:

1. Masked keys (k >= valid_len) contribute exp(-1e6) == 0 exactly, so only
   ceil(L_b/64)*64 key columns per batch need any compute.  The host gathers
   just the valid key columns; the graph is specialized at runtime to the
   valid_lens actually received (all 8 cores share one graph; per-core
   variation is data only).

2. Layout h-on-partitions with TWO queries packed per 128-partition ACT call
   (partitions 0:64 = h for q-slot-A, 64:128 = h for q-slot-B, against a kp2
   tile holding kp.T duplicated in both halves).  The "+qp" broadcast add is
   free via ACT's per-partition bias operand.  The h-reduction runs on the PE:
   pair j multiplies with a (128,32) weight whose only nonzero columns are 2j
   (top-half wv) and 2j+1 (bottom-half wv), accumulating into a shared PSUM
   tile -> 16 pairs build 32 score rows at no extra PE streaming cost.

3. All matmul operands are bf16 (fp32 streams at half rate on the PE; PSUM
   accumulation stays fp32).  ACT is dtype-independent so tanh/exp lose
   nothing; precision impact ~1e-3 total.

Softmax needs no max-subtraction (|score| <= ||wv||_1 ~ 2.6) and no on-device
normalization: an appended ones-column in the values matrix yields sum(exp)
via the same value matmul, and the host divides.  Each query lives on exactly
one core (core c owns q[64c:64c+64) of every batch), so no merge is needed.
"""

import math

import numpy as np

B, Q, KK, D, H = 4, 512, 2048, 256, 64
P = 128
G = 64            # key gather granularity (columns)
CH = 128          # value/transpose chunk granularity
QPC = 64          # queries per (core, batch)
NCORES = 8
DPAD = D + 8      # values cols + [ones, 7*zero] padding

_GRAPH_CACHE: dict = {}


def _widths(n64s):
    Ws = [n * G for n in n64s]
    offs = np.concatenate([[0], np.cumsum(Ws)]).astype(int)
    kch = [(w + CH - 1) // CH for w in Ws]           # value chunks per region
    vbase = np.concatenate([[0], np.cumsum(kch)]).astype(int)
    return Ws, offs, kch, vbase


def _build_graph(n64s, repeat=1):
    """Build + compile the single-core SPMD graph for per-batch 64-col counts.

    repeat > 1 wraps the whole body in a hardware For_i loop — used only for
    wall-clock HW timing (per-iteration delta), never for actual results.
    """
    import contextlib

    import concourse.bass as bass
    import concourse.mybir as mybir
    from concourse import bacc
    from concourse.tile import TileContext

    f32 = mybir.dt.float32
    bf16 = mybir.dt.bfloat16
    AF = mybir.ActivationFunctionType
    Ws, offs, kch, vbase = _widths(n64s)
    C = int(sum(Ws))
    assert C > 0
    totvn = int(vbase[-1])
    regions = [b for b in range(B) if n64s[b] > 0]
    last_b = regions[-1]

    nc = bacc.Bacc("TRN2", target_bir_lowering=False, debug=False)
    # wqk2: cols 0:64 = Wq, 64:192 = [Wk|Wk]; wvsid: cols 0:512 = wv2s flat,
    # 512:640 = identity.  Combined tensors keep the serialized per-DMA issue
    # cost (~0.6us each) off the critical path.
    qw_e = nc.declare_dram_parameter("qw", [D, 4 * QPC + H + P], bf16,
                                     isOutput=False)
    kT_e = nc.declare_dram_parameter("keysT", [D, C], bf16, isOutput=False)
    va_e = nc.declare_dram_parameter("vaug", [totvn * CH, DPAD], bf16,
                                     isOutput=False)
    wvsid_e = nc.declare_dram_parameter("wvsid", [P, 16 * 32 + P], bf16,
                                        isOutput=False)
    out_e = nc.declare_dram_parameter("out", [P, 4 * 3 * QPC], f32, isOutput=True)

    with TileContext(nc) as tc:
        with (
            tc.tile_pool(name="const", bufs=1) as cpool,
            tc.tile_pool(name="big", bufs=1) as kpool,
            tc.tile_pool(name="feat", bufs=3) as fpool,
            tc.tile_pool(name="xsum", bufs=2) as xpool,
            tc.tile_pool(name="pexp", bufs=2) as ppool,
            tc.tile_pool(name="pts", bufs=4) as ptpool,
            tc.tile_pool(name="ps_mm", bufs=1, space="PSUM") as ps_mm,
            tc.tile_pool(name="ps_sc", bufs=1, space="PSUM") as ps_sc,
            tc.tile_pool(name="ps_tr", bufs=1, space="PSUM") as ps_tr,
            tc.tile_pool(name="ps_v", bufs=1, space="PSUM") as ps_v,
            tc.For_i(0, repeat, 1) if repeat > 1 else contextlib.nullcontext(),
        ):
            # ---- input loads, critical-path first
            # keysT in two tiles: region 0's cols land fast so kp2 chunk 0
            # (and the first tanh) start early; the rest is one big DMA.
            w1 = min(Ws[regions[0]], C)
            kt_a = kpool.tile([P, 2, w1], bf16)
            nc.sync.dma_start(kt_a[:], kT_e[:, 0:w1].rearrange("(c p) k -> p c k", p=P))
            # queries + [Wq | Wk | Wk] arrive as one combined per-core DMA
            qw_t = cpool.tile([P, 2, 4 * QPC + H + P], bf16)
            nc.sync.dma_start(qw_t[:], qw_e[:].rearrange("(c p) q -> p c q", p=P))
            qt_t = qw_t[:, :, 0:4 * QPC]
            wqk2_t = qw_t[:, :, 4 * QPC:]
            wvs_t = cpool.tile([P, 16, 32], bf16)
            nc.sync.dma_start(wvs_t[:], wvsid_e[:, 0:512].rearrange(
                "p (j c) -> p j c", j=16))
            kt_b = None
            if C > w1:
                kt_b = kpool.tile([P, 2, C - w1], bf16)
                nc.sync.dma_start(kt_b[:],
                                  kT_e[:, w1:C].rearrange("(c p) k -> p c k", p=P))
            id_t = cpool.tile([P, P], bf16)
            nc.sync.dma_start(id_t[:], wvsid_e[:, 512:512 + P])
            va_t = kpool.tile([P, totvn, DPAD], bf16)
            nc.sync.dma_start(va_t[:], va_e[:].rearrange("(n p) d -> p n d", p=P))

            # ---- kp2 = Wk2.T @ keysT  -> (128, C) in SBUF (both halves = kp.T)
            # 512-col slices are emitted lazily, right before the first region
            # that reads them, so PE score matmuls are not queued behind the
            # whole projection.
            kp2 = kpool.tile([P, C], bf16)
            qp2 = cpool.tile([P, QPC // 2 * 4], f32)

            def emit_kp2_chunk(ps_pool, c0, w, tag="kp"):
                if c0 < w1:
                    r0, r1 = kt_a[:, 0, c0:c0 + w], kt_a[:, 1, c0:c0 + w]
                else:
                    r0 = kt_b[:, 0, c0 - w1:c0 - w1 + w]
                    r1 = kt_b[:, 1, c0 - w1:c0 - w1 + w]
                pt = ps_pool.tile([P, 512], f32, tag=tag)
                nc.tensor.matmul(pt[:, :w], lhsT=wqk2_t[:, 0, H:H + P],
                                 rhs=r0, start=True, stop=False)
                nc.tensor.matmul(pt[:, :w], lhsT=wqk2_t[:, 1, H:H + P],
                                 rhs=r1, start=False, stop=True)
                nc.vector.tensor_copy(kp2[:, c0:c0 + w], pt[:, :w])

            # ---- per-batch regions
            # PSUM budget (8 banks): kp0 1 + kp 1 + sc 4 + tr/qp 1 + v 1
            if True:
                for cc in range(0, w1, 512):
                    emit_kp2_chunk(ps_mm, cc, min(512, w1 - cc), tag="kp0")

                # qp2 bias tile; pair j of batch b = (q_{64b+j}, q_{64b+32+j});
                # the two strided copies run on ACT, which idles before the
                # tanh stream anyway (keeps DVE off the critical path)
                qps = ps_tr.tile([H, 4 * QPC], f32, tag="tr")
                nc.tensor.matmul(qps[:], lhsT=wqk2_t[:, 0, 0:H], rhs=qt_t[:, 0, :],
                                 start=True, stop=False)
                nc.tensor.matmul(qps[:], lhsT=wqk2_t[:, 1, 0:H], rhs=qt_t[:, 1, :],
                                 start=False, stop=True)
                qps_r = qps[:].rearrange("h (b c) -> h b c", b=B)
                qp2_r = qp2[:].rearrange("p (b c) -> p b c", b=B)
                nc.scalar.copy(qp2_r[0:H], qps_r[:, :, 0:32])
                nc.scalar.copy(qp2_r[H:P], qps_r[:, :, 32:QPC])

                pre_xg = None
                for b in regions:
                    W = Ws[b]
                    off = int(offs[b])
                    nch = kch[b]
                    sc = ps_sc.tile([QPC, W], f32, tag="sc")
                    # 4 pairs per tanh instruction: DVE precomputes kp2+qp for
                    # each pair (tensor_scalar_add with a per-partition scalar,
                    # 4x mode) so one wide ACT call amortizes the ~224-cycle
                    # per-instruction constant over 4 pairs.
                    NPAIR = 4
                    def emit_adds(bb, kgrp):
                        Wb = Ws[bb]
                        ob = int(offs[bb])
                        xg = xpool.tile([P, NPAIR * Wb], bf16, tag="xg")
                        for jj in range(NPAIR):
                            pi = NPAIR * kgrp + jj
                            nc.vector.tensor_scalar_add(
                                xg[:, jj * Wb:(jj + 1) * Wb], kp2[:, ob:ob + Wb],
                                qp2[:, 32 * bb + pi:32 * bb + pi + 1])
                        return xg
                    for kgrp in range(32 // NPAIR):
                        xg = pre_xg if (kgrp == 0 and pre_xg is not None)                             else emit_adds(b, kgrp)
                        pre_xg = None
                        ftg = fpool.tile([P, NPAIR * W], bf16, tag="ft")
                        nc.scalar.activation(ftg[:], xg[:], AF.Tanh)
                        for jj in range(NPAIR):
                            pi = NPAIR * kgrp + jj
                            g, j = pi // 16, pi % 16
                            for c0 in range(0, W, 512):
                                w = min(512, W - c0)
                                nc.tensor.matmul(
                                    sc[32 * g:32 * g + 32, c0:c0 + w],
                                    lhsT=wvs_t[:, j, :],
                                    rhs=ftg[:, jj * W + c0:jj * W + c0 + w],
                                    start=(j == 0), stop=(j == 15))
                    # hoist the NEXT region's first adds ahead of this
                    # region's value chain so DVE has them ready for ACT
                    bnext = regions[regions.index(b) + 1]                         if b != last_b else None
                    if bnext is not None:
                        for cc in range(int(offs[bnext]),
                                        int(offs[bnext]) + Ws[bnext], 512):
                            emit_kp2_chunk(ps_mm, cc,
                                           min(512, int(offs[bnext]) + Ws[bnext] - cc))
                        pre_xg = emit_adds(bnext, 0)
                    pe = ppool.tile([QPC, W], bf16, tag="P")
                    if b == last_b:
                        # chunk-wise exp lets the tail chain start earlier
                        for ci in range(nch):
                            cw = min(CH, W - CH * ci)
                            nc.scalar.activation(pe[:, CH * ci:CH * ci + cw],
                                                 sc[:, CH * ci:CH * ci + cw], AF.Exp)
                    else:
                        nc.scalar.activation(pe[:], sc[:], AF.Exp)
                    # one PSUM bank holds all value accumulators:
                    # cols 0:64 = values[:,0:128].T @ P, 64:128 = values[:,128:256].T @ P,
                    # 128:192 (partitions 0:8) = [ones|pad].T @ P
                    vp = ps_v.tile([P, 3 * QPC], f32, tag="v")
                    for ci in range(nch):
                        cw = min(CH, W - CH * ci)
                        tp = ps_tr.tile([P, QPC], bf16, tag="tr")
                        nc.tensor.transpose(tp[0:cw, :], pe[:, CH * ci:CH * ci + cw],
                                            id_t[0:QPC, 0:QPC])
                        ptt = ptpool.tile([P, QPC], bf16, tag="pt")
                        nc.vector.tensor_copy(ptt[0:cw, :], tp[0:cw, :])
                        gi = int(vbase[b]) + ci
                        # start=True clears has_written for the WHOLE bank, so
                        # only the very first matmul into this bank may use it;
                        # later first-writes rely on overwrite-where-bit-clear.
                        st, sp = (ci == 0), (ci == nch - 1)
                        nc.tensor.matmul(vp[:, 0:QPC], lhsT=va_t[0:cw, gi, 0:128],
                                         rhs=ptt[0:cw, :], start=st, stop=sp,
                                         skip_group_check=True)
                        nc.tensor.matmul(vp[:, QPC:2 * QPC],
                                         lhsT=va_t[0:cw, gi, 128:256],
                                         rhs=ptt[0:cw, :], start=False, stop=sp,
                                         skip_group_check=True)
                        nc.tensor.matmul(vp[0:8, 2 * QPC:3 * QPC],
                                         lhsT=va_t[0:cw, gi, 256:264],
                                         rhs=ptt[0:cw, :], start=False, stop=sp,
                                         skip_group_check=True)
                    # stream this region's whole accumulator out as one DMA;
                    # the last region's copy runs on ACT (free by then) to cut
                    # the cross-engine hop out of the tail
                    stg = ptpool.tile([P, 3 * QPC], f32, tag="stg")
                    if b == last_b:
                        nc.scalar.copy(stg[:], vp[:])
                    else:
                        nc.vector.tensor_copy(stg[:], vp[:])
                    nc.sync.dma_start(out_e[:, 3 * QPC * b:3 * QPC * (b + 1)], stg[:])

    nc.compile()
    return nc


def _prep_maps(queries, keys, values, L, Wq, Wk, wv, n64s):
    import ml_dtypes
    bf = ml_dtypes.bfloat16
    Ws, offs, kch, vbase = _widths(n64s)
    C = int(sum(Ws))
    keysT = np.zeros((D, C), np.float32)
    vaug = np.zeros((int(vbase[-1]) * CH, DPAD), np.float32)
    for b in range(B):
        if n64s[b] == 0:
            continue
        n = Ws[b]
        lb = int(L[b])
        off = int(offs[b])
        voff = int(vbase[b]) * CH
        kb = keys[b, :n, :].copy()
        vb = values[b, :n, :].copy()
        kb[lb:] = 0.0
        vb[lb:] = 0.0
        keysT[:, off:off + n] = kb.T
        vaug[voff:voff + n, 0:D] = vb
        vaug[voff:voff + n, D] = (np.arange(n) < lb).astype(np.float32)
    wv2s = np.zeros((P, 16, 32), np.float32)
    for j in range(16):
        wv2s[0:H, j, 2 * j] = wv
        wv2s[H:P, j, 2 * j + 1] = wv
    wqk2 = np.concatenate([Wq, Wk, Wk], axis=1)            # (256, 192)
    wvsid = np.concatenate([wv2s.reshape(P, 512), np.eye(P, dtype=np.float32)],
                           axis=1)                          # (128, 640)
    keysT = keysT.astype(bf)
    vaug = vaug.astype(bf)
    wvsid = np.ascontiguousarray(wvsid.astype(bf))
    in_maps = []
    for c in range(NCORES):
        qw = np.zeros((D, 4 * QPC + H + P), np.float32)
        for b in range(B):
            qw[:, QPC * b:QPC * (b + 1)] = queries[b, c * QPC:(c + 1) * QPC, :].T
        qw[:, 4 * QPC:] = wqk2
        in_maps.append(dict(qw=np.ascontiguousarray(qw.astype(bf)), keysT=keysT,
                            vaug=vaug, wvsid=wvsid))
    return in_maps


def kernel(queries, keys, values, valid_lens, Wq, Wk, wv):
    from concourse.bass_utils import run_bass_kernel_spmd

    queries = np.ascontiguousarray(np.asarray(queries, dtype=np.float32))
    keys = np.ascontiguousarray(np.asarray(keys, dtype=np.float32))
    values = np.ascontiguousarray(np.asarray(values, dtype=np.float32))
    Wq = np.ascontiguousarray(np.asarray(Wq, dtype=np.float32))
    Wk = np.ascontiguousarray(np.asarray(Wk, dtype=np.float32))
    wv = np.ascontiguousarray(np.asarray(wv, dtype=np.float32))
    L = np.clip(np.asarray(valid_lens).astype(np.int64), 0, KK)
    n64s = tuple(int(math.ceil(int(l) / G)) for l in L)

    out = np.zeros((B, Q, D), np.float32)
    for b in range(B):
        if n64s[b] == 0:
            # softmax over all -1e6 scores is uniform over ALL keys
            out[b, :, :] = values[b].mean(axis=0)[None, :]

    if sum(n64s) > 0:
        if n64s not in _GRAPH_CACHE:
            _GRAPH_CACHE[n64s] = _build_graph(n64s)
        nc = _GRAPH_CACHE[n64s]
        in_maps = _prep_maps(queries, keys, values, L, Wq, Wk, wv, n64s)
        res = run_bass_kernel_spmd(nc, in_maps, list(range(NCORES)))
        perm = np.array([32 * (r % 2) + r // 2 for r in range(QPC)])
        for c in range(NCORES):
            o = res.results[c]["out"]  # (128, 4*192): per region [d0|d1|sums]
            for b in range(B):
                if n64s[b] == 0:
                    continue
                blk = o[:, 3 * QPC * b:3 * QPC * (b + 1)]   # (128, 192)
                vals = np.concatenate([blk[:, 0:QPC], blk[:, QPC:2 * QPC]],
                                      axis=0)                # (256, 64)
                sums = blk[0, 2 * QPC:3 * QPC]               # (64,)
                out[b, c * QPC + perm, :] = (vals / sums[None, :]).T
    return out
